# revision 21
# baseline (speedup 1.0000x reference)
"""[SP-only DMA variant] ARMA2d Trainium2 kernel: conv3x3 (256->256) + per-channel circular AR
solve, data-parallel over batch across 8 NeuronCores.

Math: y = conv3x3(x, w); per channel c: out[b,c] = Gh[c] @ y[b,c] @ Gw[c].T
with Gh/Gw 64x64 circulant-inverse matrices (precomputed host-side in f64).

Device dataflow (per core, 4 images):
- conv: 18 shifted fp16 matmuls (2 ci-tiles x 9 taps) into PSUM per
  8-row block; output channels permuted parity-major (s = par*64 + pr)
  so the full-image fp16 y stores as two clean 3D DMAs per (cot, b)
  into y2[h, pair, b, parity, w].
- AR per channel-pair p (c0=2p, c1=2p+1):
    T1: one batched XBAR DMA-transpose DRAM->SBUF:
        t1s[(par,w), b, h] = y2[h, p, b, par, w]
    mm1: block-diag GwT pair (K=128): p2[(par,j), b, h]
    T2: one batched XBAR DMA-transpose SBUF->SBUF:
        t2s[(bl,h), bp, (par,j)]  (b = 2*bp + bl)
    mm2 x4 (ci x bl): lhsT = GhT[c] (K=64), rhs partition-offset slice
        -> vt[i, ci, bp, bl, j]
    out: one DMA per channel: [64h, 4b, 64w] f32.
No PE transposes, no gather DMAs.
"""
import sys
import numpy as np

if "/opt/trn_rl_repo" not in sys.path:
    sys.path.insert(0, "/opt/trn_rl_repo")

B, C, H, W = 32, 256, 64, 64
NCORES = 8
BP = B // NCORES  # images per core
NP = C // 2       # channel pairs

_CACHE = {}
LAST_EXEC_NS = None


def _build_nc():
    from contextlib import ExitStack
    import concourse.tile as tile
    from concourse import mybir, bacc

    f32 = mybir.dt.float32
    DT = mybir.dt.float16

    nc = bacc.Bacc("TRN2", target_bir_lowering=False, debug=False,
                   num_devices=NCORES)
    xp_p = nc.declare_dram_parameter("xp", [BP, 2, 128, 66, 66], DT, isOutput=False)
    wt_p = nc.declare_dram_parameter("wt", [128, 3, 3, 2, 2, 128], DT, isOutput=False)
    gwt_p = nc.declare_dram_parameter("gwt", [128, NP, 128], DT, isOutput=False)
    ghd_p = nc.declare_dram_parameter("ghd", [128, C, 128], DT, isOutput=False)
    out_p = nc.declare_dram_parameter("out", [BP, C, H, W], f32, isOutput=True)

    with tile.TileContext(nc) as tc, ExitStack() as ctx:
        consts = ctx.enter_context(tc.tile_pool(name="consts", bufs=1))
        w_sb = consts.tile([128, 3, 3, 2, 2, 128], DT)
        nc.sync.dma_start(w_sb[:], wt_p[:])
        gwt_sb = consts.tile([128, NP, 128], DT)
        nc.sync.dma_start(gwt_sb[:], gwt_p[:])
        gpool = ctx.enter_context(tc.tile_pool(name="gpool", bufs=2))

        dram = ctx.enter_context(tc.tile_pool(name="dram", bufs=1, space="DRAM"))
        # y2[h, pair, b, parity, w]
        y2 = dram.tile([H, NP, BP, 2, W], DT)

        xpool = ctx.enter_context(tc.tile_pool(name="xpool", bufs=1))
        ypool = ctx.enter_context(tc.tile_pool(name="ypool", bufs=2))
        arp = ctx.enter_context(tc.tile_pool(name="arp", bufs=4))

        # resident x tiles (both cit, all b)
        xts = []
        for b in range(BP):
            row = []
            for cit in range(2):
                xt = xpool.tile([128, 66, 66], DT, name=f"x_{b}_{cit}")
                nc.sync.dma_start(xt[:], xp_p[b, cit])
                row.append(xt)
            xts.append(row)

        cpsum = ctx.enter_context(tc.tile_pool(name="cpsum", bufs=3, space="PSUM"))
        psA = ctx.enter_context(tc.tile_pool(name="psA", bufs=2, space="PSUM"))
        psB = ctx.enter_context(tc.tile_pool(name="psB", bufs=2, space="PSUM"))

        def conv_group(cot):
            for b in range(BP):
                ysb = ypool.tile([128, H, W], DT, tag="ysb", name=f"ysb_{cot}_{b}")
                for rb in range(8):
                    ps = cpsum.tile([128, 8, 64], f32, tag="cps",
                                    name=f"ps_{cot}_{b}_{rb}")
                    k = 0
                    for cit in range(2):
                        for ky in range(3):
                            for kx in range(3):
                                lhsT = w_sb[:, ky, kx, cit, cot, :]
                                rhs = xts[b][cit][:, rb * 8 + ky: rb * 8 + ky + 8,
                                                  kx: kx + 64]
                                nc.tensor.matmul(ps[:], lhsT, rhs,
                                                 start=(k == 0), stop=(k == 17))
                                k += 1
                    dst = ysb[:, rb * 8:(rb + 1) * 8, :]
                    if rb % 2 == 0:
                        nc.vector.tensor_copy(dst, ps[:])
                    else:
                        nc.scalar.copy(dst, ps[:])
                # two stores per (cot, b): parity-major partitions
                for par in range(2):
                    nc.sync.dma_start(
                        y2[:, cot * 64:(cot + 1) * 64, b, par, :]
                        .transpose([1, 0, 2]),
                        ysb[par * 64:(par + 1) * 64])

        ghd_sb = None

        def ar_group(gg):
            # one group = 2 pairs = channels 4gg .. 4gg+3
            # T1: DRAM->SBUF batched transpose (8 groups of 128)
            t1s = arp.tile([128, 2, BP, 64], DT, tag="t1s", name=f"t1s_{gg}")
            nc.sync.dma_start_transpose(t1s[:], y2[:, 2 * gg:2 * gg + 2])
            # mm1 per pair: block-diag GwT; shared psum tile
            p2 = psA.tile([128, 2, BP, 64], f32, tag="p2", name=f"p2_{gg}")
            for pr in range(2):
                nc.tensor.matmul(p2[:, pr], gwt_sb[:, 2 * gg + pr, :],
                                 t1s[:, pr], start=True, stop=True)
            p2s = arp.tile([128, 2, BP, 64], DT, tag="p2s", name=f"p2s_{gg}")
            nc.vector.tensor_copy(p2s[:], p2[:])
            # T2: SBUF->SBUF batched transpose; partitions (bl,h),
            # groups (pr, bp), free-within (par, j)
            t2s = arp.tile([128, 2, 2, 128], DT, tag="t2s", name=f"t2s_{gg}")
            nc.sync.dma_start_transpose(t2s[:], p2s[:])
            # mm2 per (pr, par): K=128 block-diag-over-bl GhT
            vt = psB.tile([128, 2, 2, 2, 64], f32, tag="vt", name=f"vt_{gg}")
            for pr in range(2):
                for par in range(2):
                    cc = 4 * gg + 2 * pr + par
                    nc.tensor.matmul(
                        vt[:, pr, par],
                        ghd_sb[:, cc % 64, :],
                        t2s[:, pr, :, par * 64:(par + 1) * 64],
                        start=True, stop=True)
            vs = arp.tile([128, 2, 2, 2, 64], f32, tag="vs", name=f"vs_{gg}")
            nc.vector.tensor_copy(vs[:], vt[:])
            # out: per image, [64h, 4c, 64w]; b = 2*bp + bl
            for b in range(BP):
                bl, bp = b % 2, b // 2
                eng = nc.sync if b % 2 == 0 else nc.scalar
                eng.dma_start(
                    out_p[b, 4 * gg:4 * gg + 4].transpose([1, 0, 2]),
                    vs[bl * 64:(bl + 1) * 64, :, :, bp, :])

        conv_group(0)
        conv_group(1)
        for gg in range(NP // 2):
            if gg % 16 == 0:
                chunk = gg // 16
                ghd_sb = gpool.tile([128, 64, 128], DT, tag="ghd",
                                    name=f"ghd_{chunk}")
                nc.sync.dma_start(
                    ghd_sb[:], ghd_p[:, chunk * 64:(chunk + 1) * 64, :])
            ar_group(gg)

    nc.compile()
    return nc


def _host_prep(x, w, alpha):
    f16 = np.float16

    # circulant-inverse matrices from alpha (float64 for stability)
    s = np.sin(-np.pi / 4.0)
    c = np.cos(-np.pi / 4.0)
    aw = np.zeros((2, 3), dtype=np.float64)
    aw[0, 0] = np.float32(c)
    aw[1, 0] = np.float32(-s)
    aw[0, -1] = np.float32(s)
    aw[1, -1] = np.float32(c)
    at = np.tanh(alpha.astype(np.float64))  # (C,1,2,2)
    A_xy = np.einsum("ckab,bj->ckaj", at, aw)
    A_xy[:, :, :, 1] = 1.0
    A_x = A_xy[:, 0, 0, :]  # taps along H
    A_y = A_xy[:, 0, 1, :]  # taps along W

    def circ_inv(taps):
        a = np.zeros((taps.shape[0], H), dtype=np.float64)
        a[:, -1] = taps[:, 0]
        a[:, 0] = taps[:, 1]
        a[:, 1] = taps[:, 2]
        F = np.fft.fft(a, axis=-1)
        g = np.real(np.fft.ifft(1.0 / F, axis=-1))
        idx = (np.arange(H)[:, None] - np.arange(H)[None, :]) % H
        return g[:, idx]  # (C,64,64): out = G @ y

    Gh = circ_inv(A_x)
    Gw = circ_inv(A_y)
    # gwt: block-diag pairs [128=(par,w), NP, 128=(par,j)], gwt[w, p, j] = Gw[c][j, w]
    gwT = Gw.transpose(0, 2, 1)   # [c][w, j]
    gwt = np.zeros((128, NP, 128), dtype=np.float64)
    for p in range(NP):
        gwt[0:64, p, 0:64] = gwT[2 * p]
        gwt[64:128, p, 64:128] = gwT[2 * p + 1]
    # ghd: block-diag over bl [128=(bl,h), C, 128=(bl,i)],
    # ghd[bl*64+h, c, bl*64+i] = Gh[c][i, h]
    ghT = Gh.transpose(0, 2, 1)   # [c][h, i]
    ghd = np.zeros((128, C, 128), dtype=np.float64)
    for cc in range(C):
        ghd[0:64, cc, 0:64] = ghT[cc]
        ghd[64:128, cc, 64:128] = ghT[cc]

    # weights: [ci_l, ky, kx, cit, cot, co_slot]; output channel slot
    # s = par*64 + pr maps to local channel cl = pr*2 + par
    wt = w.reshape(2, 128, 2, 128, 3, 3).transpose(3, 4, 5, 2, 0, 1)
    # wt axes now [ci_l, ky, kx, cit, cot, co_l]; permute co_l -> slots
    perm = np.empty(128, dtype=np.int64)
    for s_ in range(128):
        par, pr = divmod(s_, 64)
        perm[s_] = pr * 2 + par
    wt = np.ascontiguousarray(wt[..., perm])

    # x shards, padded
    xr = x.reshape(NCORES, BP, 2, 128, H, W)
    xpad = np.zeros((NCORES, BP, 2, 128, 66, 66), dtype=np.float32)
    xpad[..., 1:65, 1:65] = xr

    return (xpad.astype(f16), wt.astype(f16), gwt.astype(f16),
            ghd.astype(f16))


def kernel(x, w, alpha, _trace=False):
    global LAST_EXEC_NS
    from concourse.bass_utils import run_bass_kernel_spmd

    x = np.ascontiguousarray(np.asarray(x), dtype=np.float32)
    w = np.ascontiguousarray(np.asarray(w), dtype=np.float32)
    alpha = np.asarray(alpha).astype(np.float64)

    key = "nc"
    if key not in _CACHE:
        _CACHE[key] = _build_nc()
    nc = _CACHE[key]

    xpad, wt, gwt, ghd = _host_prep(x, w, alpha)
    in_maps = [
        {"xp": xpad[i], "wt": wt, "gwt": gwt, "ghd": ghd}
        for i in range(NCORES)
    ]
    res = run_bass_kernel_spmd(nc, in_maps, core_ids=list(range(NCORES)),
                               trace=_trace)
    LAST_EXEC_NS = res.exec_time_ns
    _CACHE["last_res"] = res
    outs = [np.asarray(r["out"]) for r in res.results]
    return np.concatenate(outs, axis=0)


# revision 22
# speedup vs baseline: 1.3108x; 1.3108x over previous
"""ARMA2d Trainium2 kernel: conv3x3 (256->256) + per-channel circular AR
solve, data-parallel over batch across 8 NeuronCores.

Math: y = conv3x3(x, w); per channel c: out[b,c] = Gh[c] @ y[b,c] @ Gw[c].T
with Gh/Gw 64x64 circulant-inverse matrices (precomputed host-side in f64).

Device dataflow (per core, 4 images):
- conv: 18 shifted fp16 matmuls (2 ci-tiles x 9 taps) into PSUM per
  8-row block; output channels permuted parity-major (slot = par*64+pr)
  so the full-image fp16 y stores as two 3D DMAs per (cot, b) into
  y2[h, pair, b, parity, w] (stores on GpSimd SWDGE).
- AR in blocks of 4 groups (8 channel-pairs, 16 channels):
    T1: one batched XBAR DMA-transpose DRAM->SBUF (32 groups of 128):
        t1s[(par,w), (pairL, b), h]
    mm1 per pair: block-diag GwT (K=128): p2[(par,j), b, h]
    T2 per 2 groups: batched XBAR SBUF->SBUF transpose:
        t2s[(bl,h), (g2,pr,bp), (par,j)]  (b = 2*bp + bl)
    mm2 per (group, pr, par): K=128 block-diag-over-bl GhT
        -> vt[(bl,i), pr, par, bp, j]
    out: 4 DMAs per block (one per image): [64h, 16c, 64w] f32.
All HWDGE DMAs on the SP (sync) queue only — mixing SP+ACT HWDGE
breaks the scheduler's DMA-semaphore threshold counting (cross-engine
completions alias the same DMAHW sems; races under multi-core load).
No PE transposes, no gather DMAs.
"""
import sys
import numpy as np

if "/opt/trn_rl_repo" not in sys.path:
    sys.path.insert(0, "/opt/trn_rl_repo")

B, C, H, W = 32, 256, 64, 64
NCORES = 8
BP = B // NCORES  # images per core
NP = C // 2       # channel pairs
NBLK = NP // 8    # AR blocks (8 pairs = 4 groups = 16 channels each)

_CACHE = {}
LAST_EXEC_NS = None


def _build_nc():
    from contextlib import ExitStack
    import concourse.tile as tile
    from concourse import mybir, bacc

    f32 = mybir.dt.float32
    DT = mybir.dt.float16

    nc = bacc.Bacc("TRN2", target_bir_lowering=False, debug=False,
                   num_devices=NCORES)
    xp_p = nc.declare_dram_parameter("xp", [BP, 2, 128, 66, 66], DT, isOutput=False)
    wt_p = nc.declare_dram_parameter("wt", [128, 3, 3, 2, 2, 128], DT, isOutput=False)
    gwt_p = nc.declare_dram_parameter("gwt", [128, NP, 128], DT, isOutput=False)
    ghd_p = nc.declare_dram_parameter("ghd", [128, C, 128], DT, isOutput=False)
    out_p = nc.declare_dram_parameter("out", [BP, C, H, W], f32, isOutput=True)

    with tile.TileContext(nc) as tc, ExitStack() as ctx:
        consts = ctx.enter_context(tc.tile_pool(name="consts", bufs=1))
        w_sb = consts.tile([128, 3, 3, 2, 2, 128], DT)
        nc.sync.dma_start(w_sb[:], wt_p[:])
        gwt_sb = consts.tile([128, NP, 128], DT)
        nc.sync.dma_start(gwt_sb[:], gwt_p[:])
        gpool = ctx.enter_context(tc.tile_pool(name="gpool", bufs=2))

        dram = ctx.enter_context(tc.tile_pool(name="dram", bufs=1, space="DRAM"))
        # y2[h, pair, b, parity, w]
        y2 = dram.tile([H, NP, BP, 2, W], DT)

        xpool = ctx.enter_context(tc.tile_pool(name="xpool", bufs=1))
        ypool = ctx.enter_context(tc.tile_pool(name="ypool", bufs=2))
        arp = ctx.enter_context(tc.tile_pool(name="arp", bufs=3))

        # resident x tiles (both cit, all b)
        xts = []
        for b in range(BP):
            row = []
            for cit in range(2):
                xt = xpool.tile([128, 66, 66], DT, name=f"x_{b}_{cit}")
                nc.sync.dma_start(xt[:], xp_p[b, cit])
                row.append(xt)
            xts.append(row)

        cpsum = ctx.enter_context(tc.tile_pool(name="cpsum", bufs=2, space="PSUM"))
        psA = ctx.enter_context(tc.tile_pool(name="psA", bufs=2, space="PSUM"))
        psB = ctx.enter_context(tc.tile_pool(name="psB", bufs=2, space="PSUM"))

        def conv_group(cot):
            for b in range(BP):
                ysb = ypool.tile([128, H, W], DT, tag="ysb", name=f"ysb_{cot}_{b}")
                for rb in range(8):
                    ps = cpsum.tile([128, 8, 64], f32, tag="cps",
                                    name=f"ps_{cot}_{b}_{rb}")
                    k = 0
                    for cit in range(2):
                        for ky in range(3):
                            for kx in range(3):
                                lhsT = w_sb[:, ky, kx, cit, cot, :]
                                rhs = xts[b][cit][:, rb * 8 + ky: rb * 8 + ky + 8,
                                                  kx: kx + 64]
                                nc.tensor.matmul(ps[:], lhsT, rhs,
                                                 start=(k == 0), stop=(k == 17))
                                k += 1
                    dst = ysb[:, rb * 8:(rb + 1) * 8, :]
                    if rb % 2 == 0:
                        nc.vector.tensor_copy(dst, ps[:])
                    else:
                        nc.scalar.copy(dst, ps[:])
                # two stores per (cot, b) on SWDGE: parity-major partitions
                for par in range(2):
                    nc.gpsimd.dma_start(
                        y2[:, cot * 64:(cot + 1) * 64, b, par, :]
                        .transpose([1, 0, 2]),
                        ysb[par * 64:(par + 1) * 64])

        ghd_sb = None

        def ar_block(k):
            # block k: pairs 8k..8k+7, groups 4k..4k+3, channels 16k..16k+15
            # T1: one transpose for the whole block (32 groups of 128)
            t1s = arp.tile([128, 8, BP, 64], DT, tag="t1s", name=f"t1s_{k}")
            nc.sync.dma_start_transpose(t1s[:], y2[:, 8 * k:8 * k + 8])
            vs4 = arp.tile([128, 4, 2, 2, 2, 64], f32, tag="vs4", bufs=2,
                           name=f"vs4_{k}")
            for i in range(2):  # halves: 2 groups each
                p2 = psA.tile([128, 2, 2, BP, 64], f32, tag="p2",
                              name=f"p2_{k}_{i}")
                for g2 in range(2):
                    for pr in range(2):
                        pl = 4 * i + 2 * g2 + pr  # pair-within-block
                        nc.tensor.matmul(p2[:, g2, pr],
                                         gwt_sb[:, 8 * k + pl, :],
                                         t1s[:, pl], start=True, stop=True)
                p2s = arp.tile([128, 2, 2, BP, 64], DT, tag="p2s",
                               name=f"p2s_{k}_{i}")
                if i == 0:
                    nc.vector.tensor_copy(p2s[:], p2[:])
                else:
                    nc.scalar.copy(p2s[:], p2[:])
                # T2: [128,(g2,pr,b,h)=1024] -> [128(bl,h), (g2,pr,bp)=8, (par,j)]
                t2s = arp.tile([128, 2, 2, 2, 128], DT, tag="t2s",
                               name=f"t2s_{k}_{i}")
                nc.sync.dma_start_transpose(t2s[:], p2s[:])
                for g2 in range(2):
                    gg = 4 * k + 2 * i + g2
                    vt = psB.tile([128, 2, 2, 2, 64], f32, tag="vt",
                                  name=f"vt_{gg}")
                    for pr in range(2):
                        for par in range(2):
                            cc = 4 * gg + 2 * pr + par
                            nc.tensor.matmul(
                                vt[:, pr, par],
                                ghd_sb[:, cc % 32, :],
                                t2s[:, g2, pr, :, par * 64:(par + 1) * 64],
                                start=True, stop=True)
                    dst = vs4[:, 2 * i + g2]
                    if g2 == 0:
                        nc.vector.tensor_copy(dst, vt[:])
                    else:
                        nc.scalar.copy(dst, vt[:])
            # out: 4 DMAs, one per image: [64h, 16c, 64w]
            for b in range(BP):
                bl, bp = b % 2, b // 2
                nc.sync.dma_start(
                    out_p[b, 16 * k:16 * k + 16].transpose([1, 0, 2]),
                    vs4[bl * 64:(bl + 1) * 64, :, :, :, bp, :])

        for cot in range(2):
            conv_group(cot)
            for k in range(cot * (NBLK // 2), (cot + 1) * (NBLK // 2)):
                if k % 2 == 0:
                    chunk = k // 2
                    ghd_sb = gpool.tile([128, 32, 128], DT, tag="ghd",
                                        name=f"ghd_{chunk}")
                    nc.sync.dma_start(
                        ghd_sb[:], ghd_p[:, chunk * 32:(chunk + 1) * 32, :])
                ar_block(k)

    nc.compile()
    return nc


def _host_prep(x, w, alpha):
    f16 = np.float16

    # circulant-inverse matrices from alpha (float64 for stability)
    s = np.sin(-np.pi / 4.0)
    c = np.cos(-np.pi / 4.0)
    aw = np.zeros((2, 3), dtype=np.float64)
    aw[0, 0] = np.float32(c)
    aw[1, 0] = np.float32(-s)
    aw[0, -1] = np.float32(s)
    aw[1, -1] = np.float32(c)
    at = np.tanh(alpha.astype(np.float64))  # (C,1,2,2)
    A_xy = np.einsum("ckab,bj->ckaj", at, aw)
    A_xy[:, :, :, 1] = 1.0
    A_x = A_xy[:, 0, 0, :]  # taps along H
    A_y = A_xy[:, 0, 1, :]  # taps along W

    def circ_inv(taps):
        a = np.zeros((taps.shape[0], H), dtype=np.float64)
        a[:, -1] = taps[:, 0]
        a[:, 0] = taps[:, 1]
        a[:, 1] = taps[:, 2]
        F = np.fft.fft(a, axis=-1)
        g = np.real(np.fft.ifft(1.0 / F, axis=-1))
        idx = (np.arange(H)[:, None] - np.arange(H)[None, :]) % H
        return g[:, idx]  # (C,64,64): out = G @ y

    Gh = circ_inv(A_x)
    Gw = circ_inv(A_y)
    # gwt: block-diag pairs [128=(par,w), NP, 128=(par,j)], gwt[w, p, j] = Gw[c][j, w]
    gwT = Gw.transpose(0, 2, 1)   # [c][w, j]
    gwt = np.zeros((128, NP, 128), dtype=np.float64)
    for p in range(NP):
        gwt[0:64, p, 0:64] = gwT[2 * p]
        gwt[64:128, p, 64:128] = gwT[2 * p + 1]
    # ghd: block-diag over bl [128=(bl,h), C, 128=(bl,i)],
    # ghd[bl*64+h, c, bl*64+i] = Gh[c][i, h]
    ghT = Gh.transpose(0, 2, 1)   # [c][h, i]
    ghd = np.zeros((128, C, 128), dtype=np.float64)
    for cc in range(C):
        ghd[0:64, cc, 0:64] = ghT[cc]
        ghd[64:128, cc, 64:128] = ghT[cc]

    # weights: [ci_l, ky, kx, cit, cot, co_slot]; output channel slot
    # s = par*64 + pr maps to local channel cl = pr*2 + par
    wt = w.reshape(2, 128, 2, 128, 3, 3).transpose(3, 4, 5, 2, 0, 1)
    perm = np.empty(128, dtype=np.int64)
    for s_ in range(128):
        par, pr = divmod(s_, 64)
        perm[s_] = pr * 2 + par
    wt = np.ascontiguousarray(wt[..., perm])

    # x shards, padded
    xr = x.reshape(NCORES, BP, 2, 128, H, W)
    xpad = np.zeros((NCORES, BP, 2, 128, 66, 66), dtype=np.float32)
    xpad[..., 1:65, 1:65] = xr

    return (xpad.astype(f16), wt.astype(f16), gwt.astype(f16),
            ghd.astype(f16))


def kernel(x, w, alpha, _trace=False):
    global LAST_EXEC_NS
    from concourse.bass_utils import run_bass_kernel_spmd

    x = np.ascontiguousarray(np.asarray(x), dtype=np.float32)
    w = np.ascontiguousarray(np.asarray(w), dtype=np.float32)
    alpha = np.asarray(alpha).astype(np.float64)

    key = "nc"
    if key not in _CACHE:
        _CACHE[key] = _build_nc()
    nc = _CACHE[key]

    xpad, wt, gwt, ghd = _host_prep(x, w, alpha)
    in_maps = [
        {"xp": xpad[i], "wt": wt, "gwt": gwt, "ghd": ghd}
        for i in range(NCORES)
    ]
    res = run_bass_kernel_spmd(nc, in_maps, core_ids=list(range(NCORES)),
                               trace=_trace)
    LAST_EXEC_NS = res.exec_time_ns
    _CACHE["last_res"] = res
    outs = [np.asarray(r["out"]) for r in res.results]
    return np.concatenate(outs, axis=0)
